# revision 20
# baseline (speedup 1.0000x reference)
"""Causal multi-head attention on 8 Trainium2 NeuronCores.

Problem: B=2, S=2048, H=1024, NH=16, HD=64, fp32 in/out.
Sharding: tensor-parallel over heads (2 heads/core) + AllToAll to exchange
attention context so every core computes the output projection for its own
512-token slice. The matmul path runs in bf16 (inputs converted on host;
PSUM accumulation stays fp32).

Key design decisions:
  - All operands arrive HOST-pre-transposed (hsT/wqT/wkT/wvT/woT), so the
    device does plain strided DMA loads (360 GB/s bus) instead of XBAR
    DMA-transposes (14ns/tile): the x feed drops from 3.6us to 2.9us per
    chunk and Wo from 7.2us to 5.8us, and chunk 0/1 stream in per-H-tile
    pieces so the first projection matmul issues at ~3us.
  - S^T tiles live in 2-bank PSUM pairs [128, 2, 512]: full (non-diagonal)
    k-tile pairs get ONE merged exp instruction (halving ACT's ~185ns
    per-instruction overhead), diagonal tiles keep per-tile exps + 0/1
    upper-tri mask on DVE.
  - V and out-proj biases are PRE-FILLED into PSUM by DVE (tensor_copy of a
    partition-broadcast bias image), so no rank-1 PE bias matmuls remain.
  - Q/K biases fold into the PSUM->SBUF copy on DVE (tensor_scalar_add).
  - Head-0 attention for all chunks runs in phase L1 together with QKV and
    the first MOVE chunks of head-1 (their exps use L1's idle ACT time,
    shortening the ACT-bound L2 phase so that X0 still hides under it).
  - Next-chunk QKV matmuls are interleaved into attention's exp-wait slots
    via a generator (_Filler), keeping the PE queue dense through L1.
  - The ctxa head-0 gather is emitted AFTER the last chunk's ctx stores, so
    its SP-SEQ hold (waiting on X0) no longer delays the X1 launch by ~4us.
  - The head-1 ctxa gather arrives as four 2-block pieces so E's first
    accumulation steps overlap the tail of the transfer.
  - A warm-keeper block of no-op rank-1 PE matmuls bridges the X1 window so
    the tensor engine stays at full p-state until E's operands land.

Schedule per core c (heads 2c, 2c+1 = channels 128c..128c+127):
  A.  wqT + chunk-0 x (per-H-tile pieces) + biases + wkT/wvT + chunk-1 x.
  L1. Per 512-token chunk: project qT/kT [chan, tok] (+bias on DVE), V
      natural [tok, chan] -> v1 blocks [V_h0 | 1 | V_h1 | 1], head-0
      attention (and head-1 for chunks < MOVE):
        S^T[k, q] = K^T.T @ Q^T in k-tile pairs (diagonal tiles narrowed),
        P = exp(S^T/8) on ACT -> bf16 (merged per pair when both full),
        ctx[65, 512] += V1.T @ P   (row 64 = softmax denominator),
        normalize: DVE reciprocal + GPSIMD partition-broadcast + DVE mul.
  X0. AllToAll of head-0 ctx (bf16, hides under L2).
  W.  Wo load (4 pieces, DMA work during L2).
  L2. Head-1 attention for chunks MOVE..7 (broadcast via PE ones-matmul
      while the X0 collective is in flight), X1, warm-keeper.
  E.  out[t, o] = ctx.T @ WoT (+bo via PSUM prefill), per-token-tile 2-bank
      PSUM, PSUM->SBUF copies alternating ACT/DVE, one DMA out per token
      tile; host concatenates the 8 per-core slices.
"""
import sys

if '/opt/trn_rl_repo' not in sys.path:
    sys.path.insert(0, '/opt/trn_rl_repo')

import numpy as np
import ml_dtypes

import concourse.bacc as bacc
import concourse.bass as bass
import concourse.mybir as mybir
from concourse.tile import TileContext
from concourse.bass_utils import run_bass_kernel_spmd
from concourse.masks import make_upper_triangular

F32 = mybir.dt.float32
F32R = mybir.dt.float32r
BF16 = mybir.dt.bfloat16
EXP = mybir.ActivationFunctionType.Exp

B, S, H, NH, HD = 2, 2048, 1024, 16, 64
NC = 8
T = B * S                 # 4096 tokens
TC = 512                  # tokens per chunk
NCHUNK = T // TC          # 8
NTT = T // 128            # 32 token tiles
HT = H // 128             # 8 H-tiles
SCALE = 1.0 / np.sqrt(HD)

_cache = {}

AHEAD = 1                 # S-pair lookahead in L1
AHEADL2 = 1               # S-pair lookahead in L2
MOVE = 3
NWARM = 184
NWARM0 = 6
FILLN = 4
PBUFS = 3
RBUFS = 2
CSBUFS = 3
OBUFS = 3
XBUFS = 3
WORKBUFS = 2


class _Filler:
    """Holds a generator of filler instruction groups (next-chunk QKV
    matmuls). Attention calls fill() between its own PE ops so the PE
    queue never drains while waiting on ACT exps."""

    def __init__(self):
        self.it = None

    def fill(self, n=1):
        if self.it is None:
            return
        for _ in range(n):
            try:
                next(self.it)
            except StopIteration:
                self.it = None
                return

    def drain(self):
        if self.it is not None:
            for _ in self.it:
                pass
            self.it = None


def _attention(nc, pc, qpool, qT, kT, v1, ones_r, cmask01, a2a_in, ch, h,
               use_pb=True, filler=None, ahead=None):
    """Head-h causal attention for token chunk ch; writes ctx to a2a_in.

    S^T tiles are computed in PAIRS into 2-bank PSUM tiles [128, 2, 512];
    pairs of full (non-diagonal) k-tiles share one merged exp instruction.
    V1 blocks are [V_h0 | 1 | V_h1 | 1] (width 130): head h uses cols
    [65h : 65h+65] = (V_h | ones), so ctx lands in rows 0:64 and the softmax
    denominator in row 64. Normalization: DVE reciprocal of row 64, GPSIMD
    partition-broadcast (L1) or PE ones-matmul broadcast (L2, while the X0
    collective is in flight), DVE multiply -> bf16 staging -> DMA.
    """
    b, lc = ch // 4, ch % 4
    nkt = 4 * lc + 4
    npair = nkt // 2
    ctx_ps = qpool.tile([128, 512], F32, tag='ctx', bufs=2, name='ctx')

    def col0(kt):
        s = kt - 4 * lc
        return 128 * s if s >= 0 else 0

    sts = {}

    def emit_s_pair(pr):
        st = qpool.tile([128, 2, 512], F32, tag='st2', bufs=2, name='st')
        for j in (0, 1):
            kt = 2 * pr + j
            g = 16 * b + kt
            c0 = col0(kt)
            nc.tensor.matmul(
                st[:, j, c0:512],
                kT[64 * h:64 * (h + 1), 128 * g:128 * (g + 1)],
                qT[64 * h:64 * (h + 1), TC * ch + c0:TC * (ch + 1)],
                start=True, stop=True)
        sts[pr] = st

    if ahead is None:
        ahead = AHEAD
    for j in range(min(ahead + 1, npair)):
        emit_s_pair(j)
    idx = 0
    for pr in range(npair):
        st = sts.pop(pr)
        p2 = pc.tile([128, 2, 512], BF16, tag='p', bufs=PBUFS, name='p')
        both_full = (2 * pr + 1) < 4 * lc
        if both_full:
            nc.scalar.activation(p2[:], st[:], EXP, scale=float(SCALE))
        else:
            for j in (0, 1):
                kt = 2 * pr + j
                c0 = col0(kt)
                nc.scalar.activation(p2[:, j, c0:512], st[:, j, c0:512],
                                     EXP, scale=float(SCALE))
                if kt - 4 * lc >= 0:
                    nc.vector.tensor_mul(p2[:, j, c0:c0 + 128],
                                         p2[:, j, c0:c0 + 128], cmask01[:])
        if filler is not None:
            filler.fill(FILLN)
        if pr + ahead + 1 < npair:
            emit_s_pair(pr + ahead + 1)
        for j in (0, 1):
            kt = 2 * pr + j
            g = 16 * b + kt
            c0 = col0(kt)
            nc.tensor.matmul(
                ctx_ps[0:65, c0:512],
                v1[:, 130 * g + 65 * h:130 * g + 65 * h + 65],
                p2[:, j, c0:512],
                start=(idx == 0), stop=(idx == nkt - 1))
            idx += 1
    recip_f = pc.tile([1, 512], F32, tag='recip_f', bufs=RBUFS, name='recip_f')
    nc.vector.reciprocal(recip_f[:], ctx_ps[64:65, :])
    if use_pb:
        # GPSIMD broadcast — only safe while no collective occupies Pool
        bc_t = pc.tile([64, 512], F32, tag='bc_sb', bufs=RBUFS, name='bc_sb')
        nc.gpsimd.partition_broadcast(bc_t[:], recip_f[:])
        bc_sb = bc_t[:]
    else:
        recip_r = pc.tile([1, 512], F32R, tag='recip_r', bufs=2, name='recip_r')
        nc.vector.tensor_copy(recip_r[:], recip_f[:])
        bc = qpool.tile([128, 512], F32, tag='work', bufs=WORKBUFS, name='bc')
        nc.tensor.matmul(bc[0:64, :], ones_r[0:1, 0:64], recip_r[:],
                         start=True, stop=True)
        bc_sb = pc.tile([64, 512], F32, tag='bc_sb', bufs=RBUFS, name='bc_sb')
        nc.vector.tensor_copy(bc_sb[:], bc[0:64, :])
        bc_sb = bc_sb[:]
    ctx_sb = pc.tile([64, 512], BF16, tag='ctx_sb', bufs=CSBUFS, name='ctx_sb')
    nc.vector.tensor_mul(ctx_sb[:], ctx_ps[0:64, :], bc_sb)
    nc.sync.dma_start(a2a_in[ch, :, :], ctx_sb[:])


def _build(phases='ALWE'):
    key = ('nc', phases)
    if key in _cache:
        return _cache[key]
    nc = bacc.Bacc('TRN2', target_bir_lowering=False, debug=False, num_devices=NC)

    hst_d = nc.dram_tensor('hst', [NCHUNK, 128, HT * TC], BF16,
                           kind='ExternalInput')
    wq_d = nc.dram_tensor('wq', [128, HT * 128], BF16, kind='ExternalInput')
    wk_d = nc.dram_tensor('wk', [128, HT * 128], BF16, kind='ExternalInput')
    wv_d = nc.dram_tensor('wv', [128, HT * 128], BF16, kind='ExternalInput')
    wo_d = nc.dram_tensor('wo', [128, HT * H], BF16, kind='ExternalInput')
    bq_d = nc.dram_tensor('bq', [128, 1], F32, kind='ExternalInput')
    bk_d = nc.dram_tensor('bk', [128, 1], F32, kind='ExternalInput')
    bv_d = nc.dram_tensor('bvt', [1, 512], F32, kind='ExternalInput')
    bo_d = nc.dram_tensor('bo', [1, H], F32, kind='ExternalInput')
    out_d = nc.dram_tensor('out', [TC, H], F32, kind='ExternalOutput')

    with TileContext(nc) as tc:
        with tc.tile_pool(name='persist', bufs=1) as pp, \
             tc.tile_pool(name='scr', bufs=1) as sc, \
             tc.tile_pool(name='dram', bufs=1, space='DRAM') as dpool, \
             tc.tile_pool(name='psum', bufs=1, space='PSUM') as qpool:

            def ptile(shape, dt, tag):
                return pp.tile(shape, dt, tag=tag, name=tag)

            # junk first on the DVE queue: the startup warm block reads it,
            # so the PE can start at ~0.5us
            junk = ptile([1, 512], BF16, 'junk')
            nc.vector.memset(junk[:], 1.0)
            ones_f = ptile([128, 512], F32, 'ones_f')
            nc.vector.memset(ones_f[:], 1.0)
            ones_r = ptile([1, 512], F32R, 'ones_r')
            nc.vector.tensor_copy(ones_r[:], ones_f[0:1, :])
            ut_f = ptile([128, 128], F32, 'ut_f')
            make_upper_triangular(nc, ut_f[:], val=1.0, diag=True)
            ut01 = ptile([128, 128], BF16, 'ut01')
            nc.vector.tensor_copy(ut01[:], ut_f[:])

            wqT = ptile([128, HT, 128], BF16, 'wqT')
            wkT = ptile([128, HT, 128], BF16, 'wkT')
            wvT = ptile([128, HT, 128], BF16, 'wvT')
            woT = ptile([128, HT, H], BF16, 'woT')
            qT = ptile([128, T], BF16, 'qT')
            kT = ptile([128, T], BF16, 'kT')
            v1 = ptile([128, NTT * 130], BF16, 'v1')
            bv_bc = ptile([128, 512], F32, 'bv_bc')
            bo_bc = ptile([128, H], F32, 'bo_bc')
            a2a_in0 = dpool.tile([NCHUNK, 64, TC], BF16)
            a2a_out0 = dpool.tile([NCHUNK, 64, TC], BF16)
            a2a_in1 = dpool.tile([NCHUNK, 64, TC], BF16)
            a2a_out1 = dpool.tile([NCHUNK, 64, TC], BF16)

            def load_xt(ch):
                xT = sc.tile([128, HT, TC], BF16, tag='xT', bufs=XBUFS, name='xT')
                nc.sync.dma_start(xT[:].rearrange('p h c -> p (h c)'),
                                  hst_d[ch, :, :])
                return xT

            # DMA issue order matters (the DMA engines are a serial
            # resource): wq first, then chunk-0 x streamed per H-tile so the
            # first projection starts as early as possible; wk/wv right
            # after so the K projection is never weight-starved.
            nc.sync.dma_start(wqT[:].rearrange('p h c -> p (h c)'), wq_d[:])
            bv_f = ptile([1, 512], F32, 'bv_f')
            nc.sync.dma_start(bv_f[:], bv_d[:])
            xT0 = sc.tile([128, HT, TC], BF16, tag='xT', bufs=XBUFS, name='xT')
            for ht in range(HT):
                nc.sync.dma_start(xT0[:, ht, :],
                                  hst_d[0, :, TC * ht:TC * (ht + 1)])
            xts = {0: xT0}
            if 'A' in phases:
                for w_src, w_dst in ((wk_d, wkT), (wv_d, wvT)):
                    nc.sync.dma_start(w_dst[:].rearrange('p h c -> p (h c)'),
                                      w_src[:])
            bq_col = ptile([128, 1], F32, 'bq_col')
            bk_col = ptile([128, 1], F32, 'bk_col')
            nc.sync.dma_start(bq_col[:], bq_d[:])
            nc.sync.dma_start(bk_col[:], bk_d[:])
            bo_f = ptile([1, H], F32, 'bo_f')
            nc.sync.dma_start(bo_f[:], bo_d[:])
            nc.gpsimd.partition_broadcast(bv_bc[:], bv_f[:])
            nc.gpsimd.partition_broadcast(bo_bc[:], bo_f[:])
            xT1 = sc.tile([128, HT, TC], BF16, tag='xT', bufs=XBUFS, name='xT')
            nc.sync.dma_start(xT1[:, 0:4, :].rearrange('p h c -> p (h c)'),
                              hst_d[1, :, 0:4 * TC])
            nc.sync.dma_start(xT1[:, 4:8, :].rearrange('p h c -> p (h c)'),
                              hst_d[1, :, 4 * TC:8 * TC])
            xts[1] = xT1

            # Startup warm block: no-op matmuls keep the PE busy from t~0.5us
            # until the first xT/wT DMAs land, so the first projection
            # matmuls are priced at a ramped clock.
            warm0 = qpool.tile([128, 512], F32, tag='ctx', bufs=2,
                               name='warm0')
            for i in range(NWARM0):
                nc.tensor.matmul(warm0[:], junk[0:1, 0:128], junk[0:1, :],
                                 start=(i == 0), stop=(i == NWARM0 - 1))

            # v1 ones columns (col 64 of each 65-block), one strided memset
            ones_dst = bass.AP(v1.tensor, v1.offset + 64,
                               [list(v1.ap[0]), [65, 2 * NTT]])
            nc.vector.memset(ones_dst, 1.0)

            def proj_ops(ch):
                """QKV projections for chunk ch into qT/kT/v1, as a
                generator yielding every PE matmul so attention can
                interleave this work into its exp-wait slots."""
                xT = xts.pop(ch)
                for w_t, b_c, dst in ((wqT, bq_col, qT), (wkT, bk_col, kT)):
                    ps = qpool.tile([128, 512], F32, tag='work',
                                    bufs=WORKBUFS, name='work')
                    for ht in range(HT):
                        nc.tensor.matmul(
                            ps[:], w_t[:, ht, :], xT[:, ht, :],
                            start=(ht == 0), stop=(ht == HT - 1))
                        yield
                    nc.vector.tensor_scalar_add(
                        dst[:, TC * ch:TC * (ch + 1)], ps[:], b_c[:])
                    yield
                # V in natural [tok, chan] layout: 4 col-slice accum groups
                # in one PSUM bank, bias PRE-FILLED by DVE (start=False)
                ps = qpool.tile([128, 512], F32, tag='work', bufs=WORKBUFS,
                                name='work')
                nc.vector.tensor_copy(ps[:], bv_bc[:])
                for tt in range(4):
                    cs = slice(128 * tt, 128 * (tt + 1))
                    for ht in range(HT):
                        nc.tensor.matmul(
                            ps[:, cs], xT[:, ht, cs], wvT[:, ht, :],
                            start=False, stop=(ht == HT - 1),
                            skip_group_check=True)
                        yield
                    kt = 4 * ch + tt
                    base = 130 * kt
                    # [V_h0 | gap | V_h1]: one strided copy fills cols
                    # base..base+63 and base+65..base+128
                    dst = bass.AP(v1.tensor, v1.offset + base,
                                  [list(v1.ap[0]), [65, 2], [1, 64]])
                    nc.vector.tensor_copy(
                        dst, ps[:, cs].rearrange('p (g c) -> p g c', g=2))
                    yield

            # ---- L1: QKV + head-0 attention for all chunks, plus the first
            # MOVE chunks of head-1 (their exps fill L1's idle ACT time;
            # this shortens the ACT-bound L2 phase so X0 still hides) ----
            filler = _Filler()
            if 'L' in phases:
                for _ in proj_ops(0):
                    pass
                for ch in range(NCHUNK):
                    if ch + 2 < NCHUNK:
                        xts[ch + 2] = load_xt(ch + 2)
                    if ch + 1 < NCHUNK:
                        filler.it = proj_ops(ch + 1)
                    _attention(nc, sc, qpool, qT, kT, v1, ones_r, ut01,
                               a2a_in0, ch, 0, filler=filler)
                    filler.drain()
                # MOVE chunks of head-1 run at the END of L1: their S/PV
                # matmuls (qT/kT long ready, deep prefetch) densify the
                # late-L1 window where the QKV filler has run dry, and
                # their exps still land in L1's idle ACT time.
                for ch in range(MOVE):
                    _attention(nc, sc, qpool, qT, kT, v1, ones_r, ut01,
                               a2a_in1, ch, 1, ahead=2)

                # ---- X0: AllToAll for head 0 (overlaps L2) ----
                nc.gpsimd.collective_compute(
                    'AllToAll', mybir.AluOpType.bypass,
                    replica_groups=[list(range(NC))],
                    ins=[a2a_in0[:]], outs=[a2a_out0[:]],
                )

            # ---- W: Wo load in 4 pieces (DMA work during L1/L2, split so
            # no single transfer holds the DMA engines too long) ----
            if 'W' in phases:
                for i in range(4):
                    nc.sync.dma_start(
                        woT[:, 2 * i:2 * (i + 1), :].rearrange(
                            'p h c -> p (h c)'),
                        wo_d[:, 2 * i * H:2 * (i + 1) * H])

            ctxa = pp.tile([128, NC * TC], BF16, tag='ctxa', name='ctxa')

            # ---- L2: remaining head-1 attention ----
            if 'L' in phases:
                for ch in range(MOVE, NCHUNK):
                    _attention(nc, sc, qpool, qT, kT, v1, ones_r, ut01,
                               a2a_in1, ch, 1, use_pb=False, ahead=AHEADL2)
                # h0 half of ctxa on the POOL queue (SWDGE): the tile
                # scheduler reorders same-queue DMAs, so on SP this load's
                # wait-for-X0 would block the last ctx stores and delay the
                # X1 launch; Pool is idle here and the transfer runs during
                # X1.
                nc.gpsimd.dma_start(
                    ctxa[0:64, :].rearrange('p (i t) -> p i t', i=NC),
                    a2a_out0[:].rearrange('i p t -> p i t'))
                nc.gpsimd.collective_compute(
                    'AllToAll', mybir.AluOpType.bypass,
                    replica_groups=[list(range(NC))],
                    ins=[a2a_in1[:]], outs=[a2a_out1[:]],
                )
                # Warm-keeper: one long PE accumulation of no-op rank-1
                # matmuls into a scratch bank. It has no dependencies, so it
                # runs back-to-back from the moment L2's PE work ends until
                # roughly when X1 + the h1 ctxa DMA complete, keeping the
                # tensor engine at full p-state through the collective.
                warm = qpool.tile([128, 512], F32, tag='ctx', bufs=2,
                                  name='warm')
                for i in range(NWARM):
                    nc.tensor.matmul(warm[:], ones_r[0:1, 0:128],
                                     ones_r[0:1, :],
                                     start=(i == 0), stop=(i == NWARM - 1))

            # ---- E: output projection for my 512 tokens ----
            if 'E' in phases:
                # h1 half of ctxa in pieces on the ACT queue (idle after
                # L2): single-block leading pieces so E's first
                # accumulation steps start as early as possible, with the
                # rest streaming in just ahead of consumption
                for lo, hi in ((0, 1), (1, 2), (2, 4), (4, 6), (6, 8)):
                    nblk = hi - lo
                    dst = ctxa[64:128, TC * lo:TC * hi]
                    if nblk > 1:
                        dst = dst.rearrange('p (i t) -> p i t', i=nblk)
                        src = a2a_out1[lo:hi].rearrange('i p t -> p i t')
                    else:
                        src = a2a_out1[lo].rearrange('p t -> p t')
                    nc.scalar.dma_start(dst, src)
                for tt in range(4):
                    ps = qpool.tile([128, 2, 512], F32, tag='st2', bufs=2,
                                    name='eps')
                    for oc in range(2):
                        for it in range(NC):
                            nc.tensor.matmul(
                                ps[:, oc, :],
                                ctxa[:, TC * it + 128 * tt:
                                     TC * it + 128 * (tt + 1)],
                                woT[:, it, 512 * oc:512 * (oc + 1)],
                                start=(it == 0), stop=(it == NC - 1))
                        # per-oc PSUM->SBUF copy folds the output bias in
                        # on DVE (per-column bias, so ACT's per-partition
                        # bias can't); halving the copy/store grain trims
                        # the serial tail after the last matmul
                        o_sb = sc.tile([128, 512], F32, tag='o_sb',
                                       bufs=OBUFS, name='o_sb')
                        nc.vector.tensor_add(
                            o_sb[:], ps[:, oc, :],
                            bo_bc[:, 512 * oc:512 * (oc + 1)])
                        nc.sync.dma_start(
                            out_d[128 * tt:128 * (tt + 1),
                                  512 * oc:512 * (oc + 1)], o_sb[:])

    nc.compile()
    _cache[key] = nc
    return nc


def kernel(hidden_states, Wq, bq, Wk, bk, Wv, bv, Wo, bo, **run_kwargs):
    nc = _build()
    bf = ml_dtypes.bfloat16
    hs = np.asarray(hidden_states, np.float32).reshape(T, H).astype(bf)
    # host-side layout prep: transpose to [chunk, partition, h-tile, token]
    hsT = np.ascontiguousarray(
        hs.reshape(NCHUNK, TC, HT, 128).transpose(0, 3, 2, 1)
    ).reshape(NCHUNK, 128, HT * TC)
    Wq, Wk, Wv, Wo = (np.asarray(w, np.float32) for w in (Wq, Wk, Wv, Wo))
    bq, bk, bv, bo = (np.asarray(b, np.float32) for b in (bq, bk, bv, bo))
    woT = np.ascontiguousarray(
        Wo.T.reshape(HT, 128, H).transpose(1, 0, 2)
    ).reshape(128, HT * H).astype(bf)
    bo_row = np.ascontiguousarray(bo.reshape(1, H))

    def wt(w, r):
        # [128, HT*128]: wT[p, ht*128+m] = w[r][m, 128*ht+p]
        return np.ascontiguousarray(
            w[r].T.reshape(HT, 128, 128).transpose(1, 0, 2)
        ).reshape(128, HT * 128).astype(bf)

    in_maps = []
    for c in range(NC):
        r = slice(128 * c, 128 * (c + 1))
        in_maps.append({
            'hst': hsT,
            'wq': wt(Wq, r),
            'wk': wt(Wk, r),
            'wv': wt(Wv, r),
            'wo': woT,
            'bq': np.ascontiguousarray(bq[r].reshape(128, 1)),
            'bk': np.ascontiguousarray(bk[r].reshape(128, 1)),
            'bvt': np.ascontiguousarray(np.tile(bv[r], 4).reshape(1, 512)),
            'bo': bo_row,
        })
    res = run_bass_kernel_spmd(nc, in_maps, core_ids=list(range(NC)), **run_kwargs)
    out = np.concatenate([res.results[c]['out'] for c in range(NC)], axis=0)
    kernel.last_results = res
    return out.reshape(B, S, H)


# revision 21
# speedup vs baseline: 1.0350x; 1.0350x over previous
"""Causal multi-head attention on 8 Trainium2 NeuronCores.

Problem: B=2, S=2048, H=1024, NH=16, HD=64, fp32 in/out.
Sharding: tensor-parallel over heads (2 heads/core) + AllToAll to exchange
attention context so every core computes the output projection for its own
512-token slice. The matmul path runs in bf16 (inputs converted on host;
PSUM accumulation stays fp32).

Key design decisions:
  - All operands arrive HOST-pre-transposed (hsT/wqT/wkT/wvT/woT), so the
    device does plain strided DMA loads (360 GB/s bus) instead of XBAR
    DMA-transposes (14ns/tile): the x feed drops from 3.6us to 2.9us per
    chunk and Wo from 7.2us to 5.8us, and chunk 0/1 stream in per-H-tile
    pieces so the first projection matmul issues at ~3us.
  - S^T tiles live in 2-bank PSUM pairs [128, 2, 512]: full (non-diagonal)
    k-tile pairs get ONE merged exp instruction (halving ACT's ~185ns
    per-instruction overhead), diagonal tiles keep per-tile exps + 0/1
    upper-tri mask on DVE.
  - V and out-proj biases are PRE-FILLED into PSUM by DVE (tensor_copy of a
    partition-broadcast bias image), so no rank-1 PE bias matmuls remain.
  - Q/K biases fold into the PSUM->SBUF copy on DVE (tensor_scalar_add).
  - Head-0 attention for all chunks runs in phase L1 together with QKV and
    the first MOVE chunks of head-1 (their exps use L1's idle ACT time,
    shortening the ACT-bound L2 phase so that X0 still hides under it).
  - Next-chunk QKV matmuls are interleaved into attention's exp-wait slots
    via a generator (_Filler), keeping the PE queue dense through L1.
  - The ctxa head-0 gather is emitted AFTER the last chunk's ctx stores, so
    its SP-SEQ hold (waiting on X0) no longer delays the X1 launch by ~4us.
  - The head-1 ctxa gather arrives as four 2-block pieces so E's first
    accumulation steps overlap the tail of the transfer.
  - A warm-keeper block of no-op rank-1 PE matmuls bridges the X1 window so
    the tensor engine stays at full p-state until E's operands land.

Schedule per core c (heads 2c, 2c+1 = channels 128c..128c+127):
  A.  wqT + chunk-0 x (per-H-tile pieces) + biases + wkT/wvT + chunk-1 x.
  L1. Per 512-token chunk: project qT/kT [chan, tok] (+bias on DVE), V
      natural [tok, chan] -> v1 blocks [V_h0 | 1 | V_h1 | 1], head-0
      attention (and head-1 for chunks < MOVE):
        S^T[k, q] = K^T.T @ Q^T in k-tile pairs (diagonal tiles narrowed),
        P = exp(S^T/8) on ACT -> bf16 (merged per pair when both full),
        ctx[65, 512] += V1.T @ P   (row 64 = softmax denominator),
        normalize: DVE reciprocal + GPSIMD partition-broadcast + DVE mul.
  X0. AllToAll of head-0 ctx (bf16, hides under L2).
  W.  Wo load (4 pieces, DMA work during L2).
  L2. Head-1 attention for chunks MOVE..7 (broadcast via PE ones-matmul
      while the X0 collective is in flight), X1, warm-keeper.
  E.  out[t, o] = ctx.T @ WoT (+bo via PSUM prefill), per-token-tile 2-bank
      PSUM, PSUM->SBUF copies alternating ACT/DVE, one DMA out per token
      tile; host concatenates the 8 per-core slices.
"""
import sys

if '/opt/trn_rl_repo' not in sys.path:
    sys.path.insert(0, '/opt/trn_rl_repo')

import numpy as np
import ml_dtypes

import concourse.bacc as bacc
import concourse.bass as bass
import concourse.mybir as mybir
from concourse.tile import TileContext
from concourse.bass_utils import run_bass_kernel_spmd
from concourse.masks import make_upper_triangular

F32 = mybir.dt.float32
F32R = mybir.dt.float32r
BF16 = mybir.dt.bfloat16
EXP = mybir.ActivationFunctionType.Exp

B, S, H, NH, HD = 2, 2048, 1024, 16, 64
NC = 8
T = B * S                 # 4096 tokens
TC = 512                  # tokens per chunk
NCHUNK = T // TC          # 8
NTT = T // 128            # 32 token tiles
HT = H // 128             # 8 H-tiles
SCALE = 1.0 / np.sqrt(HD)

_cache = {}

AHEAD = 1                 # S-pair lookahead in L1
AHEADL2 = 1               # S-pair lookahead in L2
MOVE = 3
NWARM = 184
NWARM0 = 6
FILLN = 4
PBUFS = 3
RBUFS = 2
CSBUFS = 3
OBUFS = 3
XBUFS = 3
WORKBUFS = 2


class _Filler:
    """Holds a generator of filler instruction groups (next-chunk QKV
    matmuls). Attention calls fill() between its own PE ops so the PE
    queue never drains while waiting on ACT exps."""

    def __init__(self):
        self.it = None

    def fill(self, n=1):
        if self.it is None:
            return
        for _ in range(n):
            try:
                next(self.it)
            except StopIteration:
                self.it = None
                return

    def drain(self):
        if self.it is not None:
            for _ in self.it:
                pass
            self.it = None


def _attention(nc, pc, qpool, qT, kT, v1, ones_r, cmask01, a2a_in, ch, h,
               use_pb=True, filler=None, ahead=None):
    """Head-h causal attention for token chunk ch; writes ctx to a2a_in.

    S^T tiles are computed in PAIRS into 2-bank PSUM tiles [128, 2, 512];
    pairs of full (non-diagonal) k-tiles share one merged exp instruction.
    V1 blocks are [V_h0 | 1 | V_h1 | 1] (width 130): head h uses cols
    [65h : 65h+65] = (V_h | ones), so ctx lands in rows 0:64 and the softmax
    denominator in row 64. Normalization: DVE reciprocal of row 64, GPSIMD
    partition-broadcast (L1) or PE ones-matmul broadcast (L2, while the X0
    collective is in flight), DVE multiply -> bf16 staging -> DMA.
    """
    b, lc = ch // 4, ch % 4
    nkt = 4 * lc + 4
    npair = nkt // 2
    ctx_ps = qpool.tile([128, 512], F32, tag='ctx', bufs=2, name='ctx')

    def col0(kt):
        s = kt - 4 * lc
        return 128 * s if s >= 0 else 0

    sts = {}

    def emit_s_pair(pr):
        st = qpool.tile([128, 2, 512], F32, tag='st2', bufs=2, name='st')
        for j in (0, 1):
            kt = 2 * pr + j
            g = 16 * b + kt
            c0 = col0(kt)
            nc.tensor.matmul(
                st[:, j, c0:512],
                kT[64 * h:64 * (h + 1), 128 * g:128 * (g + 1)],
                qT[64 * h:64 * (h + 1), TC * ch + c0:TC * (ch + 1)],
                start=True, stop=True)
        sts[pr] = st

    if ahead is None:
        ahead = AHEAD
    for j in range(min(ahead + 1, npair)):
        emit_s_pair(j)
    idx = 0
    for pr in range(npair):
        st = sts.pop(pr)
        p2 = pc.tile([128, 2, 512], BF16, tag='p', bufs=PBUFS, name='p')
        both_full = (2 * pr + 1) < 4 * lc
        if both_full:
            nc.scalar.activation(p2[:], st[:], EXP, scale=float(SCALE))
        else:
            for j in (0, 1):
                kt = 2 * pr + j
                c0 = col0(kt)
                nc.scalar.activation(p2[:, j, c0:512], st[:, j, c0:512],
                                     EXP, scale=float(SCALE))
                if kt - 4 * lc >= 0:
                    nc.vector.tensor_mul(p2[:, j, c0:c0 + 128],
                                         p2[:, j, c0:c0 + 128], cmask01[:])
        if filler is not None:
            filler.fill(FILLN)
        if pr + ahead + 1 < npair:
            emit_s_pair(pr + ahead + 1)
        for j in (0, 1):
            kt = 2 * pr + j
            g = 16 * b + kt
            c0 = col0(kt)
            nc.tensor.matmul(
                ctx_ps[0:65, c0:512],
                v1[:, 130 * g + 65 * h:130 * g + 65 * h + 65],
                p2[:, j, c0:512],
                start=(idx == 0), stop=(idx == nkt - 1))
            idx += 1
    recip_f = pc.tile([1, 512], F32, tag='recip_f', bufs=RBUFS, name='recip_f')
    nc.vector.reciprocal(recip_f[:], ctx_ps[64:65, :])
    if use_pb:
        # GPSIMD broadcast — only safe while no collective occupies Pool
        bc_t = pc.tile([64, 512], F32, tag='bc_sb', bufs=RBUFS, name='bc_sb')
        nc.gpsimd.partition_broadcast(bc_t[:], recip_f[:])
        bc_sb = bc_t[:]
    else:
        recip_r = pc.tile([1, 512], F32R, tag='recip_r', bufs=2, name='recip_r')
        nc.vector.tensor_copy(recip_r[:], recip_f[:])
        bc = qpool.tile([128, 512], F32, tag='work', bufs=WORKBUFS, name='bc')
        nc.tensor.matmul(bc[0:64, :], ones_r[0:1, 0:64], recip_r[:],
                         start=True, stop=True)
        bc_sb = pc.tile([64, 512], F32, tag='bc_sb', bufs=RBUFS, name='bc_sb')
        nc.vector.tensor_copy(bc_sb[:], bc[0:64, :])
        bc_sb = bc_sb[:]
    ctx_sb = pc.tile([64, 512], BF16, tag='ctx_sb', bufs=CSBUFS, name='ctx_sb')
    nc.vector.tensor_mul(ctx_sb[:], ctx_ps[0:64, :], bc_sb)
    nc.sync.dma_start(a2a_in[ch, :, :], ctx_sb[:])


def _build(phases='ALWE'):
    key = ('nc', phases)
    if key in _cache:
        return _cache[key]
    nc = bacc.Bacc('TRN2', target_bir_lowering=False, debug=False, num_devices=NC)

    hst_d = nc.dram_tensor('hst', [NCHUNK, 128, HT * TC], BF16,
                           kind='ExternalInput')
    wq_d = nc.dram_tensor('wq', [128, HT * 128], BF16, kind='ExternalInput')
    wk_d = nc.dram_tensor('wk', [128, HT * 128], BF16, kind='ExternalInput')
    wv_d = nc.dram_tensor('wv', [128, HT * 128], BF16, kind='ExternalInput')
    wo_d = nc.dram_tensor('wo', [128, HT * H], BF16, kind='ExternalInput')
    bq_d = nc.dram_tensor('bq', [128, 1], F32, kind='ExternalInput')
    bk_d = nc.dram_tensor('bk', [128, 1], F32, kind='ExternalInput')
    bv_d = nc.dram_tensor('bvt', [1, 512], F32, kind='ExternalInput')
    bo_d = nc.dram_tensor('bo', [1, H], F32, kind='ExternalInput')
    out_d = nc.dram_tensor('out', [TC, H], F32, kind='ExternalOutput')

    with TileContext(nc) as tc:
        with tc.tile_pool(name='persist', bufs=1) as pp, \
             tc.tile_pool(name='scr', bufs=1) as sc, \
             tc.tile_pool(name='dram', bufs=1, space='DRAM') as dpool, \
             tc.tile_pool(name='psum', bufs=1, space='PSUM') as qpool:

            def ptile(shape, dt, tag):
                return pp.tile(shape, dt, tag=tag, name=tag)

            # junk first on the DVE queue: the startup warm block reads it,
            # so the PE can start at ~0.5us
            junk = ptile([1, 512], BF16, 'junk')
            nc.vector.memset(junk[:], 1.0)
            ones_f = ptile([128, 512], F32, 'ones_f')
            nc.vector.memset(ones_f[:], 1.0)
            ones_r = ptile([1, 512], F32R, 'ones_r')
            nc.vector.tensor_copy(ones_r[:], ones_f[0:1, :])
            ut_f = ptile([128, 128], F32, 'ut_f')
            make_upper_triangular(nc, ut_f[:], val=1.0, diag=True)
            ut01 = ptile([128, 128], BF16, 'ut01')
            nc.vector.tensor_copy(ut01[:], ut_f[:])

            wqT = ptile([128, HT, 128], BF16, 'wqT')
            wkT = ptile([128, HT, 128], BF16, 'wkT')
            wvT = ptile([128, HT, 128], BF16, 'wvT')
            woT = ptile([128, HT, H], BF16, 'woT')
            qT = ptile([128, T], BF16, 'qT')
            kT = ptile([128, T], BF16, 'kT')
            v1 = ptile([128, NTT * 130], BF16, 'v1')
            bv_bc = ptile([128, 512], F32, 'bv_bc')
            bo_bc = ptile([128, H], F32, 'bo_bc')
            a2a_in0 = dpool.tile([NCHUNK, 64, TC], BF16)
            a2a_out0 = dpool.tile([NCHUNK, 64, TC], BF16)
            a2a_in1 = dpool.tile([NCHUNK, 64, TC], BF16)
            a2a_out1 = dpool.tile([NCHUNK, 64, TC], BF16)

            def load_xt(ch):
                xT = sc.tile([128, HT, TC], BF16, tag='xT', bufs=XBUFS, name='xT')
                nc.sync.dma_start(xT[:].rearrange('p h c -> p (h c)'),
                                  hst_d[ch, :, :])
                return xT

            # DMA issue order matters (the DMA engines are a serial
            # resource): wq first, then chunk-0 x streamed per H-tile so the
            # first projection starts as early as possible; wk/wv right
            # after so the K projection is never weight-starved.
            nc.sync.dma_start(wqT[:].rearrange('p h c -> p (h c)'), wq_d[:])
            bv_f = ptile([1, 512], F32, 'bv_f')
            nc.sync.dma_start(bv_f[:], bv_d[:])
            xT0 = sc.tile([128, HT, TC], BF16, tag='xT', bufs=XBUFS, name='xT')
            for ht in range(HT):
                nc.sync.dma_start(xT0[:, ht, :],
                                  hst_d[0, :, TC * ht:TC * (ht + 1)])
            xts = {0: xT0}
            if 'A' in phases:
                for w_src, w_dst in ((wk_d, wkT), (wv_d, wvT)):
                    nc.sync.dma_start(w_dst[:].rearrange('p h c -> p (h c)'),
                                      w_src[:])
            bq_col = ptile([128, 1], F32, 'bq_col')
            bk_col = ptile([128, 1], F32, 'bk_col')
            nc.sync.dma_start(bq_col[:], bq_d[:])
            nc.sync.dma_start(bk_col[:], bk_d[:])
            bo_f = ptile([1, H], F32, 'bo_f')
            nc.sync.dma_start(bo_f[:], bo_d[:])
            nc.gpsimd.partition_broadcast(bv_bc[:], bv_f[:])
            nc.gpsimd.partition_broadcast(bo_bc[:], bo_f[:])
            xT1 = sc.tile([128, HT, TC], BF16, tag='xT', bufs=XBUFS, name='xT')
            nc.sync.dma_start(xT1[:, 0:4, :].rearrange('p h c -> p (h c)'),
                              hst_d[1, :, 0:4 * TC])
            nc.sync.dma_start(xT1[:, 4:8, :].rearrange('p h c -> p (h c)'),
                              hst_d[1, :, 4 * TC:8 * TC])
            xts[1] = xT1

            # Startup warm block: no-op matmuls keep the PE busy from t~0.5us
            # until the first xT/wT DMAs land, so the first projection
            # matmuls are priced at a ramped clock.
            warm0 = qpool.tile([128, 512], F32, tag='ctx', bufs=2,
                               name='warm0')
            for i in range(NWARM0):
                nc.tensor.matmul(warm0[:], junk[0:1, 0:128], junk[0:1, :],
                                 start=(i == 0), stop=(i == NWARM0 - 1))

            # v1 ones columns (col 64 of each 65-block), one strided memset
            ones_dst = bass.AP(v1.tensor, v1.offset + 64,
                               [list(v1.ap[0]), [65, 2 * NTT]])
            nc.vector.memset(ones_dst, 1.0)

            def proj_ops(ch):
                """QKV projections for chunk ch into qT/kT/v1, as a
                generator yielding every PE matmul so attention can
                interleave this work into its exp-wait slots."""
                xT = xts.pop(ch)
                for w_t, b_c, dst in ((wqT, bq_col, qT), (wkT, bk_col, kT)):
                    ps = qpool.tile([128, 512], F32, tag='work',
                                    bufs=WORKBUFS, name='work')
                    for ht in range(HT):
                        nc.tensor.matmul(
                            ps[:], w_t[:, ht, :], xT[:, ht, :],
                            start=(ht == 0), stop=(ht == HT - 1))
                        yield
                    nc.vector.tensor_scalar_add(
                        dst[:, TC * ch:TC * (ch + 1)], ps[:], b_c[:])
                    yield
                # V in natural [tok, chan] layout: 4 col-slice accum groups
                # in one PSUM bank, bias PRE-FILLED by DVE (start=False)
                ps = qpool.tile([128, 512], F32, tag='work', bufs=WORKBUFS,
                                name='work')
                nc.vector.tensor_copy(ps[:], bv_bc[:])
                for tt in range(4):
                    cs = slice(128 * tt, 128 * (tt + 1))
                    for ht in range(HT):
                        nc.tensor.matmul(
                            ps[:, cs], xT[:, ht, cs], wvT[:, ht, :],
                            start=False, stop=(ht == HT - 1),
                            skip_group_check=True)
                        yield
                    kt = 4 * ch + tt
                    base = 130 * kt
                    # [V_h0 | gap | V_h1]: one strided copy fills cols
                    # base..base+63 and base+65..base+128
                    dst = bass.AP(v1.tensor, v1.offset + base,
                                  [list(v1.ap[0]), [65, 2], [1, 64]])
                    nc.vector.tensor_copy(
                        dst, ps[:, cs].rearrange('p (g c) -> p g c', g=2))
                    yield

            # ---- L1: QKV + head-0 attention for all chunks, plus the first
            # MOVE chunks of head-1 (their exps fill L1's idle ACT time;
            # this shortens the ACT-bound L2 phase so X0 still hides) ----
            filler = _Filler()
            if 'L' in phases:
                for _ in proj_ops(0):
                    pass
                for ch in range(NCHUNK):
                    if ch + 2 < NCHUNK:
                        xts[ch + 2] = load_xt(ch + 2)
                    if ch + 1 < NCHUNK:
                        filler.it = proj_ops(ch + 1)
                    _attention(nc, sc, qpool, qT, kT, v1, ones_r, ut01,
                               a2a_in0, ch, 0, filler=filler)
                    if ch < MOVE:
                        _attention(nc, sc, qpool, qT, kT, v1, ones_r, ut01,
                                   a2a_in1, ch, 1, filler=filler)
                    filler.drain()

                # ---- X0: AllToAll for head 0 (overlaps L2) ----
                nc.gpsimd.collective_compute(
                    'AllToAll', mybir.AluOpType.bypass,
                    replica_groups=[list(range(NC))],
                    ins=[a2a_in0[:]], outs=[a2a_out0[:]],
                )

            # ---- W: Wo load in 4 pieces (DMA work during L1/L2, split so
            # no single transfer holds the DMA engines too long) ----
            if 'W' in phases:
                for i in range(4):
                    nc.sync.dma_start(
                        woT[:, 2 * i:2 * (i + 1), :].rearrange(
                            'p h c -> p (h c)'),
                        wo_d[:, 2 * i * H:2 * (i + 1) * H])

            ctxa = pp.tile([128, NC * TC], BF16, tag='ctxa', name='ctxa')

            # ---- L2: remaining head-1 attention ----
            if 'L' in phases:
                for ch in range(MOVE, NCHUNK):
                    _attention(nc, sc, qpool, qT, kT, v1, ones_r, ut01,
                               a2a_in1, ch, 1, use_pb=False, ahead=AHEADL2)
                # h0 half of ctxa on the POOL queue (SWDGE): the tile
                # scheduler reorders same-queue DMAs, so on SP this load's
                # wait-for-X0 would block the last ctx stores and delay the
                # X1 launch; Pool is idle here and the transfer runs during
                # X1.
                nc.gpsimd.dma_start(
                    ctxa[0:64, :].rearrange('p (i t) -> p i t', i=NC),
                    a2a_out0[:].rearrange('i p t -> p i t'))
                nc.gpsimd.collective_compute(
                    'AllToAll', mybir.AluOpType.bypass,
                    replica_groups=[list(range(NC))],
                    ins=[a2a_in1[:]], outs=[a2a_out1[:]],
                )
                # Warm-keeper: one long PE accumulation of no-op rank-1
                # matmuls into a scratch bank. It has no dependencies, so it
                # runs back-to-back from the moment L2's PE work ends until
                # roughly when X1 + the h1 ctxa DMA complete, keeping the
                # tensor engine at full p-state through the collective.
                warm = qpool.tile([128, 512], F32, tag='ctx', bufs=2,
                                  name='warm')
                for i in range(NWARM):
                    nc.tensor.matmul(warm[:], ones_r[0:1, 0:128],
                                     ones_r[0:1, :],
                                     start=(i == 0), stop=(i == NWARM - 1))

            # ---- E: output projection for my 512 tokens ----
            if 'E' in phases:
                # h1 half of ctxa in pieces on the ACT queue (idle after
                # L2): single-block leading pieces so E's first
                # accumulation steps start as early as possible, with the
                # rest streaming in just ahead of consumption
                for lo, hi in ((0, 1), (1, 2), (2, 4), (4, 6), (6, 8)):
                    nblk = hi - lo
                    dst = ctxa[64:128, TC * lo:TC * hi]
                    if nblk > 1:
                        dst = dst.rearrange('p (i t) -> p i t', i=nblk)
                        src = a2a_out1[lo:hi].rearrange('i p t -> p i t')
                    else:
                        src = a2a_out1[lo].rearrange('p t -> p t')
                    nc.scalar.dma_start(dst, src)
                for tt in range(4):
                    ps = qpool.tile([128, 2, 512], F32, tag='st2', bufs=2,
                                    name='eps')
                    for oc in range(2):
                        for it in range(NC):
                            nc.tensor.matmul(
                                ps[:, oc, :],
                                ctxa[:, TC * it + 128 * tt:
                                     TC * it + 128 * (tt + 1)],
                                woT[:, it, 512 * oc:512 * (oc + 1)],
                                start=(it == 0), stop=(it == NC - 1))
                        # per-oc PSUM->SBUF copy folds the output bias in
                        # on DVE (per-column bias, so ACT's per-partition
                        # bias can't); halving the copy/store grain trims
                        # the serial tail after the last matmul
                        o_sb = sc.tile([128, 512], F32, tag='o_sb',
                                       bufs=OBUFS, name='o_sb')
                        nc.vector.tensor_add(
                            o_sb[:], ps[:, oc, :],
                            bo_bc[:, 512 * oc:512 * (oc + 1)])
                        nc.sync.dma_start(
                            out_d[128 * tt:128 * (tt + 1),
                                  512 * oc:512 * (oc + 1)], o_sb[:])

    nc.compile()
    _cache[key] = nc
    return nc


def kernel(hidden_states, Wq, bq, Wk, bk, Wv, bv, Wo, bo, **run_kwargs):
    nc = _build()
    bf = ml_dtypes.bfloat16
    hs = np.asarray(hidden_states, np.float32).reshape(T, H).astype(bf)
    # host-side layout prep: transpose to [chunk, partition, h-tile, token]
    hsT = np.ascontiguousarray(
        hs.reshape(NCHUNK, TC, HT, 128).transpose(0, 3, 2, 1)
    ).reshape(NCHUNK, 128, HT * TC)
    Wq, Wk, Wv, Wo = (np.asarray(w, np.float32) for w in (Wq, Wk, Wv, Wo))
    bq, bk, bv, bo = (np.asarray(b, np.float32) for b in (bq, bk, bv, bo))
    woT = np.ascontiguousarray(
        Wo.T.reshape(HT, 128, H).transpose(1, 0, 2)
    ).reshape(128, HT * H).astype(bf)
    bo_row = np.ascontiguousarray(bo.reshape(1, H))

    def wt(w, r):
        # [128, HT*128]: wT[p, ht*128+m] = w[r][m, 128*ht+p]
        return np.ascontiguousarray(
            w[r].T.reshape(HT, 128, 128).transpose(1, 0, 2)
        ).reshape(128, HT * 128).astype(bf)

    in_maps = []
    for c in range(NC):
        r = slice(128 * c, 128 * (c + 1))
        in_maps.append({
            'hst': hsT,
            'wq': wt(Wq, r),
            'wk': wt(Wk, r),
            'wv': wt(Wv, r),
            'wo': woT,
            'bq': np.ascontiguousarray(bq[r].reshape(128, 1)),
            'bk': np.ascontiguousarray(bk[r].reshape(128, 1)),
            'bvt': np.ascontiguousarray(np.tile(bv[r], 4).reshape(1, 512)),
            'bo': bo_row,
        })
    res = run_bass_kernel_spmd(nc, in_maps, core_ids=list(range(NC)), **run_kwargs)
    out = np.concatenate([res.results[c]['out'] for c in range(NC)], axis=0)
    kernel.last_results = res
    return out.reshape(B, S, H)
